# revision 7
# baseline (speedup 1.0000x reference)
"""KL loss on 8 TRN2 cores — v3b: fp8 sigmas + bf16 mus, 4-engine balance.

Identity: exp(sigma_p - sigma_q) = exp(sigma_p) * exp(-sigma_q), so the KL sum
is  0.5*[ sum(sq) - sum(sp) + sum( em*(ep + d^2) ) - B*D ]
with em = exp(-sq), ep = exp(sp), d = mq - mp.

The sigma tensors travel as fp8-e4m3 (consumed only by ACT exps and PE
matmul-sums, both read fp8 natively — the DVE, which has no fast path for
1-byte dtypes, never touches them). The mu tensors travel as bf16 (DVE 2x
mode). Per-core traffic: 2x2MB fp8 + 2x4MB bf16 = 12 MiB, vs the 358 GB/s
per-NeuronCore HBM limit -> ~37us stream.

Host packs row-contiguous [ROWS, 2, D] so each 128-row tile is ONE contiguous
line per partition -> 128 DMA descriptors per transfer (4x fewer than
plane-major; DMA_DIRECT2D issue measured 3.8us -> ~1us).

Work split sized to measured engine rates (DVE 0.96 GHz w/ 2x bf16 tensor-
tensor but 1x reduce, ACT 1.2 GHz, PE 0.55us per 512-col matmul, gpsimd
~40 G elem/s; tensor_tensor_reduce does not compile on this walrus build):
  DVE   : d = mq-mp ; dd = d*d ; dd[0:W/2] += ep ; t = em*dd ;
          reduce t[:, 0:RV] -> acc_m
  gpsimd: dd[W/2:W] += ep  (the other half of the add)
  ACT   : em = exp(-sq) ; ep = exp(sp) ; Identity-accum t[:, RV:W] -> acc_a
  PE    : sum(sq) - sum(sp) via +/-1 fp8 stationaries into one [1,512] PSUM
Pipeline: 11 units (7 full 128-row tiles + 4 column-quarters of the last
tile), 6-slot ring, single HWDGE queue on SP, 2 DMAs per unit (sigma first
so ACT/PE start half a unit early).
"""

from contextlib import ExitStack

import numpy as np

import concourse.bass as bass
from concourse import mybir
from concourse.bass_utils import run_bass_kernel_spmd

B, D = 8192, 2048
NCORES = 8
ROWS = B // NCORES
P = 128
NT = ROWS // P  # 8 row-tiles
NQ = 4
NU = (NT - 1) + NQ  # 11 units
NSLOT = 6

F32 = mybir.dt.float32
BF16 = mybir.dt.bfloat16
FP8 = mybir.dt.float8e4


def _w_of(u):
    return D if u < NT - 1 else D // NQ


def _rv_of(w):
    # DVE reduce-X share of the t-sum; ACT Identity-accum takes the rest.
    return 896 if w == D else 224


def _build_nc(detect_races=True):
    nc = bass.Bass(
        trn_type="TRN2", target_bir_lowering=False,
        detect_race_conditions=detect_races,
    )

    xs = nc.dram_tensor("xs", [ROWS, 2, D], FP8, kind="ExternalInput")
    xm = nc.dram_tensor("xm", [ROWS, 2, D], BF16, kind="ExternalInput")
    out = nc.dram_tensor("out", [P, 3], F32, kind="ExternalOutput")

    Exp = mybir.ActivationFunctionType.Exp
    Identity = mybir.ActivationFunctionType.Identity
    Alu = mybir.AluOpType
    X = mybir.AxisListType.X

    ctx = ExitStack()
    with ctx:
        sslot = [
            ctx.enter_context(nc.sbuf_tensor(f"ss{k}", [P, 2 * D], FP8))
            for k in range(NSLOT)
        ]
        mslot = [
            ctx.enter_context(nc.sbuf_tensor(f"ms{k}", [P, 2 * D], BF16))
            for k in range(NSLOT)
        ]
        d_b = [ctx.enter_context(nc.sbuf_tensor(f"d{j}", [P, D], BF16)) for j in range(2)]
        dd_b = [ctx.enter_context(nc.sbuf_tensor(f"dd{j}", [P, D], BF16)) for j in range(2)]
        em_b = [ctx.enter_context(nc.sbuf_tensor(f"em{j}", [P, D], BF16)) for j in range(2)]
        ep_b = [ctx.enter_context(nc.sbuf_tensor(f"ep{j}", [P, D], BF16)) for j in range(2)]
        t_b = [ctx.enter_context(nc.sbuf_tensor(f"t{j}", [P, D], BF16)) for j in range(2)]
        acc_a = ctx.enter_context(nc.sbuf_tensor("acc_a", [P, NU], F32))
        acc_m = ctx.enter_context(nc.sbuf_tensor("acc_m", [P, NU], F32))
        res = ctx.enter_context(nc.sbuf_tensor("res", [P, 3], F32))
        ones8 = ctx.enter_context(nc.sbuf_tensor("ones8", [P, 1], FP8))
        neg8 = ctx.enter_context(nc.sbuf_tensor("neg8", [P, 1], FP8))
        psAB = ctx.enter_context(nc.psum_tensor("psAB", [P, 512], F32))

        dsem = ctx.enter_context(nc.semaphore("dsem"))
        v_sem = ctx.enter_context(nc.semaphore("v_sem"))
        a_sem = ctx.enter_context(nc.semaphore("a_sem"))
        p_sem = ctx.enter_context(nc.semaphore("p_sem"))
        g_sem = ctx.enter_context(nc.semaphore("g_sem"))
        g2_sem = ctx.enter_context(nc.semaphore("g2_sem"))
        osem = ctx.enter_context(nc.semaphore("osem"))

        def src_ap(xt, u):
            if u < NT - 1:
                # one contiguous [2*D] line per row
                return bass.AP(xt, u * P * 2 * D, [[2 * D, P], [1, 2 * D]])
            q = u - (NT - 1)
            w = D // NQ
            # per row: both planes' q-th column slice
            return bass.AP(
                xt, (NT - 1) * P * 2 * D + q * w, [[2 * D, P], [D, 2], [1, w]]
            )

        with nc.Block() as block:

            @block.sync
            def _(sync):
                for u in range(NU):
                    if u >= NSLOT:
                        pu = u - NSLOT
                        sync.wait_ge(a_sem, 3 * pu + 2)  # ACT read sigma slot
                        sync.wait_ge(p_sem, pu + 1)      # PE read sigma slot
                        sync.wait_ge(v_sem, 4 * pu + 1)  # DVE read mu slot
                    w = _w_of(u)
                    k = u % NSLOT
                    sync.dma_start(sslot[k][:, 0 : 2 * w], src_ap(xs, u)).then_inc(
                        dsem, 16
                    )
                    sync.dma_start(mslot[k][:, 0 : 2 * w], src_ap(xm, u)).then_inc(
                        dsem, 16
                    )
                sync.wait_ge(v_sem, 4 * NU + 1)  # res final
                sync.dma_start(out[:, :], res[:, :]).then_inc(osem, 16)
                sync.wait_ge(osem, 16)

            @block.vector
            def _(vector):
                for u in range(NU):
                    j, k, w = u % 2, u % NSLOT, _w_of(u)
                    ah, rv = w // 2, _rv_of(w)
                    vector.wait_ge(dsem, 32 * (u + 1))  # mu arrived (2nd DMA)
                    vector.tensor_sub(
                        d_b[j][:, 0:w], mslot[k][:, 0:w], mslot[k][:, w : 2 * w]
                    ).then_inc(v_sem, 1)
                    if u >= 2:
                        vector.wait_ge(g2_sem, u - 1)  # dd[j] free (gpsimd add u-2)
                    vector.tensor_mul(
                        dd_b[j][:, 0:w], d_b[j][:, 0:w], d_b[j][:, 0:w]
                    ).then_inc(v_sem, 1)
                    vector.wait_ge(a_sem, 3 * u + 2)  # em+ep ready
                    vector.tensor_add(
                        dd_b[j][:, 0:ah], ep_b[j][:, 0:ah], dd_b[j][:, 0:ah]
                    )
                    if u >= 2:
                        vector.wait_ge(a_sem, 3 * (u - 2) + 3)  # t[j] free (ACT accum)
                    vector.wait_ge(g2_sem, u + 1)  # gpsimd add-right done
                    vector.tensor_mul(
                        t_b[j][:, 0:w], em_b[j][:, 0:w], dd_b[j][:, 0:w]
                    ).then_inc(v_sem, 1)
                    vector.tensor_reduce(
                        acc_m[:, u : u + 1], t_b[j][:, 0:rv], axis=X, op=Alu.add
                    ).then_inc(v_sem, 1)
                vector.wait_ge(a_sem, 3 * NU)  # all ACT accums final
                vector.tensor_reduce(res[:, 0:1], acc_a[:, 0:NU], axis=X, op=Alu.add)
                vector.tensor_reduce(res[:, 1:2], acc_m[:, 0:NU], axis=X, op=Alu.add)
                vector.wait_ge(p_sem, NU)  # all PE matmuls final
                vector.tensor_reduce(
                    res[0:1, 2:3], psAB[0:1, :], axis=X, op=Alu.add
                ).then_inc(v_sem, 1)

            @block.scalar
            def _(scalar):
                for u in range(NU):
                    j, k, w = u % 2, u % NSLOT, _w_of(u)
                    rv = _rv_of(w)
                    scalar.wait_ge(dsem, 32 * u + 16)  # sigma arrived (1st DMA)
                    if u >= 2:
                        scalar.wait_ge(v_sem, 4 * (u - 2) + 3)  # em free (DVE mul-t)
                        scalar.wait_ge(g2_sem, u - 1)  # ep free (gpsimd add u-2)
                    scalar.activation(
                        em_b[j][:, 0:w], sslot[k][:, 0:w], Exp, scale=-1.0
                    ).then_inc(a_sem, 1)
                    scalar.activation(
                        ep_b[j][:, 0:w], sslot[k][:, w : 2 * w], Exp
                    ).then_inc(a_sem, 1)
                    scalar.wait_ge(v_sem, 4 * u + 3)  # t ready
                    scalar.activation(
                        t_b[j][:, rv:w], t_b[j][:, rv:w], Identity,
                        accum_out=acc_a[:, u : u + 1],
                    ).then_inc(a_sem, 1)

            @block.tensor
            def _(pe):
                pe.wait_ge(g_sem, 1)  # stationary vectors ready
                mm = 0
                for u in range(NU):
                    k, w = u % NSLOT, _w_of(u)
                    nch = max(1, w // 512)
                    cw = min(w, 512)
                    pe.wait_ge(dsem, 32 * u + 16)  # sigma arrived
                    for c in range(nch):
                        pe.matmul(
                            psAB[0:1, 0:cw], ones8[:, 0:1],
                            sslot[k][:, c * cw : (c + 1) * cw],
                            start=(mm == 0), stop=False,
                        )
                        mm += 1
                    for c in range(nch):
                        i = pe.matmul(
                            psAB[0:1, 0:cw], neg8[:, 0:1],
                            sslot[k][:, w + c * cw : w + (c + 1) * cw],
                            start=False, stop=(u == NU - 1 and c == nch - 1),
                        )
                        mm += 1
                    i.then_inc(p_sem, 1)

            @block.gpsimd
            def _(gpsimd):
                gpsimd.memset(res[:, :], 0.0)
                gpsimd.memset(ones8[:, :], 1.0)
                gpsimd.memset(neg8[:, :], -1.0).then_inc(g_sem, 1)
                for u in range(NU):
                    j, w = u % 2, _w_of(u)
                    ah = w // 2
                    gpsimd.wait_ge(a_sem, 3 * u + 2)  # ep ready
                    gpsimd.wait_ge(v_sem, 4 * u + 2)  # dd written
                    gpsimd.tensor_add(
                        dd_b[j][:, ah:w], ep_b[j][:, ah:w], dd_b[j][:, ah:w]
                    ).then_inc(g2_sem, 1)

    return nc


_NC = None


def _get_nc():
    global _NC
    if _NC is None:
        _NC = _build_nc()
    return _NC


def _pack_inputs(inputs):
    fp8 = np.dtype(mybir.dt.np(FP8))
    bf16 = np.dtype(mybir.dt.np(BF16))
    xs = np.stack(
        [
            np.asarray(inputs["sigma_q"], dtype=np.float32),
            np.asarray(inputs["sigma_p"], dtype=np.float32),
        ],
        axis=1,
    ).astype(fp8)  # [B, 2, D]
    xm = np.stack(
        [
            np.asarray(inputs["mu_q"], dtype=np.float32),
            np.asarray(inputs["mu_p"], dtype=np.float32),
        ],
        axis=1,
    ).astype(bf16)  # [B, 2, D]
    return [
        {
            "xs": np.ascontiguousarray(xs[c * ROWS : (c + 1) * ROWS]),
            "xm": np.ascontiguousarray(xm[c * ROWS : (c + 1) * ROWS]),
        }
        for c in range(NCORES)
    ]


def _run(inputs, **kw):
    return run_bass_kernel_spmd(
        _get_nc(), _pack_inputs(inputs), core_ids=list(range(NCORES)), **kw
    )


def _combine(results):
    # per core: res[:,0]=acc_a rowsums, res[:,1]=acc_m rowsums,
    #           res[0,2]=sum(sq)-sum(sp)
    tot = 0.0
    for r in results:
        o = np.asarray(r["out"], dtype=np.float64)
        tot += o[:, 0].sum() + o[:, 1].sum() + o[0, 2]
    kl = 0.5 * (tot - B * D)
    return np.asarray(kl, dtype=np.float32)


def kernel(**inputs):
    return _combine(_run(inputs).results)


def run_traced(inputs, **kw):
    br = _run(inputs, trace=True, **kw)
    return _combine(br.results), br


# revision 13
# speedup vs baseline: 1.2395x; 1.2395x over previous
"""KL loss on 8 TRN2 cores — v3b: fp8 sigmas + bf16 mus, 4-engine balance.

Identity: exp(sigma_p - sigma_q) = exp(sigma_p) * exp(-sigma_q), so the KL sum
is  0.5*[ sum(sq) - sum(sp) + sum( em*(ep + d^2) ) - B*D ]
with em = exp(-sq), ep = exp(sp), d = mq - mp.

The sigma tensors travel as fp8-e4m3 (consumed only by ACT exps and PE
matmul-sums, both read fp8 natively — the DVE, which has no fast path for
1-byte dtypes, never touches them). The mu tensors travel as bf16 (DVE 2x
mode). Per-core traffic: 2x2MB fp8 + 2x4MB bf16 = 12 MiB, vs the 358 GB/s
per-NeuronCore HBM limit -> ~37us stream.

Host packs row-contiguous [ROWS, 2, D] so each 128-row tile is ONE contiguous
line per partition -> 128 DMA descriptors per transfer (4x fewer than
plane-major; DMA_DIRECT2D issue measured 3.8us -> ~1us).

Work split sized to measured engine rates (DVE 0.96 GHz w/ 2x bf16 tensor-
tensor but 1x reduce, ACT 1.2 GHz, PE 0.55us per 512-col matmul, gpsimd
~40 G elem/s; tensor_tensor_reduce does not compile on this walrus build):
  DVE   : d = mq-mp ; dd = d*d ; dd[0:W/2] += ep ; t = em*dd ;
          reduce t[:, 0:RV] -> acc_m
  gpsimd: dd[W/2:W] += ep  (the other half of the add)
  ACT   : em = exp(-sq) ; ep = exp(sp) ; Identity-accum t[:, RV:W] -> acc_a
  PE    : sum(sq) - sum(sp) via +/-1 fp8 stationaries into one [1,512] PSUM
Pipeline: 11 units (7 full 128-row tiles + 4 column-quarters of the last
tile), 6-slot ring, single HWDGE queue on SP, 2 DMAs per unit (sigma first
so ACT/PE start half a unit early).
"""

from contextlib import ExitStack

import numpy as np

import concourse.bass as bass
from concourse import mybir
from concourse.bass_utils import run_bass_kernel_spmd

B, D = 8192, 2048
NCORES = 8
ROWS = B // NCORES
P = 128
NT = ROWS // P  # 8 row-tiles
NQ = 4
NU = (NT - 1) + NQ  # 11 units
NSLOT = 6

F32 = mybir.dt.float32
BF16 = mybir.dt.bfloat16
FP8 = mybir.dt.float8e4


def _w_of(u):
    return D if u < NT - 1 else D // NQ


def _rv_of(w):
    # DVE reduce-X share of the t-sum; ACT Identity-accum takes the rest.
    return 896 if w == D else 224


def _build_nc(detect_races=True):
    nc = bass.Bass(
        trn_type="TRN2", target_bir_lowering=False,
        detect_race_conditions=detect_races,
    )

    xs = nc.dram_tensor("xs", [ROWS, 2, D], FP8, kind="ExternalInput")
    xm = nc.dram_tensor("xm", [ROWS, 2, D], BF16, kind="ExternalInput")
    out = nc.dram_tensor("out", [P, 3], F32, kind="ExternalOutput")

    Exp = mybir.ActivationFunctionType.Exp
    Identity = mybir.ActivationFunctionType.Identity
    Alu = mybir.AluOpType
    X = mybir.AxisListType.X

    ctx = ExitStack()
    with ctx:
        sslot = [
            ctx.enter_context(nc.sbuf_tensor(f"ss{k}", [P, 2 * D], FP8))
            for k in range(NSLOT)
        ]
        mslot = [
            ctx.enter_context(nc.sbuf_tensor(f"ms{k}", [P, 2 * D], BF16))
            for k in range(NSLOT)
        ]
        d_b = [ctx.enter_context(nc.sbuf_tensor(f"d{j}", [P, D], BF16)) for j in range(2)]
        dd_b = [ctx.enter_context(nc.sbuf_tensor(f"dd{j}", [P, D], BF16)) for j in range(2)]
        em_b = [ctx.enter_context(nc.sbuf_tensor(f"em{j}", [P, D], BF16)) for j in range(2)]
        ep_b = [ctx.enter_context(nc.sbuf_tensor(f"ep{j}", [P, D], BF16)) for j in range(2)]
        t_b = [ctx.enter_context(nc.sbuf_tensor(f"t{j}", [P, D], BF16)) for j in range(2)]
        acc_a = ctx.enter_context(nc.sbuf_tensor("acc_a", [P, NU], F32))
        acc_m = ctx.enter_context(nc.sbuf_tensor("acc_m", [P, NU], F32))
        res = ctx.enter_context(nc.sbuf_tensor("res", [P, 3], F32))
        ones8 = ctx.enter_context(nc.sbuf_tensor("ones8", [P, 1], FP8))
        neg8 = ctx.enter_context(nc.sbuf_tensor("neg8", [P, 1], FP8))
        psAB = ctx.enter_context(nc.psum_tensor("psAB", [P, 512], F32))

        dsem = ctx.enter_context(nc.semaphore("dsem"))
        v_sem = ctx.enter_context(nc.semaphore("v_sem"))
        a_sem = ctx.enter_context(nc.semaphore("a_sem"))
        p_sem = ctx.enter_context(nc.semaphore("p_sem"))
        g_sem = ctx.enter_context(nc.semaphore("g_sem"))
        g2_sem = ctx.enter_context(nc.semaphore("g2_sem"))
        aa_sem = ctx.enter_context(nc.semaphore("aa_sem"))
        osem = ctx.enter_context(nc.semaphore("osem"))

        def src_ap(xt, u):
            if u < NT - 1:
                # one contiguous [2*D] line per row
                return bass.AP(xt, u * P * 2 * D, [[2 * D, P], [1, 2 * D]])
            q = u - (NT - 1)
            w = D // NQ
            # per row: both planes' q-th column slice
            return bass.AP(
                xt, (NT - 1) * P * 2 * D + q * w, [[2 * D, P], [D, 2], [1, w]]
            )

        with nc.Block() as block:

            @block.sync
            def _(sync):
                for u in range(NU):
                    if u >= NSLOT:
                        pu = u - NSLOT
                        sync.wait_ge(a_sem, 2 * pu + 2)  # ACT read sigma slot
                        sync.wait_ge(p_sem, pu + 1)      # PE read sigma slot
                        sync.wait_ge(v_sem, 4 * pu + 1)  # DVE read mu slot
                    w = _w_of(u)
                    k = u % NSLOT
                    sync.dma_start(sslot[k][:, 0 : 2 * w], src_ap(xs, u)).then_inc(
                        dsem, 16
                    )
                    sync.dma_start(mslot[k][:, 0 : 2 * w], src_ap(xm, u)).then_inc(
                        dsem, 16
                    )
                sync.wait_ge(v_sem, 4 * NU + 1)  # res final
                sync.dma_start(out[:, :], res[:, :]).then_inc(osem, 16)
                sync.wait_ge(osem, 16)

            @block.vector
            def _(vector):
                for u in range(NU):
                    j, k, w = u % 2, u % NSLOT, _w_of(u)
                    ah, rv = w // 2, _rv_of(w)
                    vector.wait_ge(dsem, 32 * (u + 1))  # mu arrived (2nd DMA)
                    vector.tensor_sub(
                        d_b[j][:, 0:w], mslot[k][:, 0:w], mslot[k][:, w : 2 * w]
                    ).then_inc(v_sem, 1)
                    if u >= 2:
                        vector.wait_ge(g2_sem, u - 1)  # dd[j] free (gpsimd add u-2)
                    vector.tensor_mul(
                        dd_b[j][:, 0:w], d_b[j][:, 0:w], d_b[j][:, 0:w]
                    ).then_inc(v_sem, 1)
                    vector.wait_ge(a_sem, 2 * u + 2)  # em+ep ready
                    vector.tensor_add(
                        dd_b[j][:, 0:ah], ep_b[j][:, 0:ah], dd_b[j][:, 0:ah]
                    )
                    if u >= 2:
                        vector.wait_ge(aa_sem, u - 1)  # t[j] free (ACT accum u-2)
                    vector.wait_ge(g2_sem, u + 1)  # gpsimd add-right done
                    vector.tensor_mul(
                        t_b[j][:, 0:w], em_b[j][:, 0:w], dd_b[j][:, 0:w]
                    ).then_inc(v_sem, 1)
                    vector.tensor_reduce(
                        acc_m[:, u : u + 1], t_b[j][:, 0:rv], axis=X, op=Alu.add
                    ).then_inc(v_sem, 1)
                vector.wait_ge(aa_sem, NU)  # all ACT accums final
                vector.tensor_reduce(res[:, 0:1], acc_a[:, 0:NU], axis=X, op=Alu.add)
                vector.tensor_reduce(res[:, 1:2], acc_m[:, 0:NU], axis=X, op=Alu.add)
                vector.wait_ge(p_sem, NU)  # all PE matmuls final
                vector.tensor_reduce(
                    res[0:1, 2:3], psAB[0:1, :], axis=X, op=Alu.add
                ).then_inc(v_sem, 1)

            @block.scalar
            def _(scalar):
                # The Identity-accum of unit u-1 runs AFTER em/ep of unit u:
                # lagging it one unit keeps the ep(u) -> gpsimd add(u) ->
                # DVE mul-t(u) -> accum(u) chain out of ACT's issue order, so
                # consecutive units pipeline instead of serializing.
                def id_accum(u):
                    j, w = u % 2, _w_of(u)
                    rv = _rv_of(w)
                    scalar.wait_ge(v_sem, 4 * u + 3)  # t(u) ready
                    scalar.activation(
                        t_b[j][:, rv:w], t_b[j][:, rv:w], Identity,
                        accum_out=acc_a[:, u : u + 1],
                    ).then_inc(aa_sem, 1)

                for u in range(NU):
                    j, k, w = u % 2, u % NSLOT, _w_of(u)
                    scalar.wait_ge(dsem, 32 * u + 16)  # sigma arrived (1st DMA)
                    if u >= 2:
                        scalar.wait_ge(v_sem, 4 * (u - 2) + 3)  # em free (DVE mul-t)
                        scalar.wait_ge(g2_sem, u - 1)  # ep free (gpsimd add u-2)
                    scalar.activation(
                        em_b[j][:, 0:w], sslot[k][:, 0:w], Exp, scale=-1.0
                    ).then_inc(a_sem, 1)
                    scalar.activation(
                        ep_b[j][:, 0:w], sslot[k][:, w : 2 * w], Exp
                    ).then_inc(a_sem, 1)
                    if u >= 1:
                        id_accum(u - 1)
                id_accum(NU - 1)

            @block.tensor
            def _(pe):
                pe.wait_ge(g_sem, 1)  # stationary vectors ready
                mm = 0
                for u in range(NU):
                    k, w = u % NSLOT, _w_of(u)
                    nch = max(1, w // 512)
                    cw = min(w, 512)
                    pe.wait_ge(dsem, 32 * u + 16)  # sigma arrived
                    for c in range(nch):
                        pe.matmul(
                            psAB[0:1, 0:cw], ones8[:, 0:1],
                            sslot[k][:, c * cw : (c + 1) * cw],
                            start=(mm == 0), stop=False,
                        )
                        mm += 1
                    for c in range(nch):
                        i = pe.matmul(
                            psAB[0:1, 0:cw], neg8[:, 0:1],
                            sslot[k][:, w + c * cw : w + (c + 1) * cw],
                            start=False, stop=(u == NU - 1 and c == nch - 1),
                        )
                        mm += 1
                    i.then_inc(p_sem, 1)

            @block.gpsimd
            def _(gpsimd):
                gpsimd.memset(res[:, :], 0.0)
                gpsimd.memset(ones8[:, :], 1.0)
                gpsimd.memset(neg8[:, :], -1.0).then_inc(g_sem, 1)
                for u in range(NU):
                    j, w = u % 2, _w_of(u)
                    ah = w // 2
                    gpsimd.wait_ge(a_sem, 2 * u + 2)  # ep ready
                    gpsimd.wait_ge(v_sem, 4 * u + 2)  # dd written
                    gpsimd.tensor_add(
                        dd_b[j][:, ah:w], ep_b[j][:, ah:w], dd_b[j][:, ah:w]
                    ).then_inc(g2_sem, 1)

    return nc


_NC = None


def _get_nc():
    global _NC
    if _NC is None:
        _NC = _build_nc()
    return _NC


def _pack_inputs(inputs):
    fp8 = np.dtype(mybir.dt.np(FP8))
    bf16 = np.dtype(mybir.dt.np(BF16))
    xs = np.stack(
        [
            np.asarray(inputs["sigma_q"], dtype=np.float32),
            np.asarray(inputs["sigma_p"], dtype=np.float32),
        ],
        axis=1,
    ).astype(fp8)  # [B, 2, D]
    xm = np.stack(
        [
            np.asarray(inputs["mu_q"], dtype=np.float32),
            np.asarray(inputs["mu_p"], dtype=np.float32),
        ],
        axis=1,
    ).astype(bf16)  # [B, 2, D]
    return [
        {
            "xs": np.ascontiguousarray(xs[c * ROWS : (c + 1) * ROWS]),
            "xm": np.ascontiguousarray(xm[c * ROWS : (c + 1) * ROWS]),
        }
        for c in range(NCORES)
    ]


def _run(inputs, **kw):
    return run_bass_kernel_spmd(
        _get_nc(), _pack_inputs(inputs), core_ids=list(range(NCORES)), **kw
    )


def _combine(results):
    # per core: res[:,0]=acc_a rowsums, res[:,1]=acc_m rowsums,
    #           res[0,2]=sum(sq)-sum(sp)
    tot = 0.0
    for r in results:
        o = np.asarray(r["out"], dtype=np.float64)
        tot += o[:, 0].sum() + o[:, 1].sum() + o[0, 2]
    kl = 0.5 * (tot - B * D)
    return np.asarray(kl, dtype=np.float32)


def kernel(**inputs):
    return _combine(_run(inputs).results)


def run_traced(inputs, **kw):
    br = _run(inputs, trace=True, **kw)
    return _combine(br.results), br


# revision 25
# speedup vs baseline: 1.3958x; 1.1261x over previous
"""KL loss on 8 TRN2 cores — v3b: fp8 sigmas + bf16 mus, 4-engine balance.

Identity: exp(sigma_p - sigma_q) = exp(sigma_p) * exp(-sigma_q), so the KL sum
is  0.5*[ sum(sq) - sum(sp) + sum( em*(ep + d^2) ) - B*D ]
with em = exp(-sq), ep = exp(sp), d = mq - mp.

The sigma tensors travel as fp8-e4m3 (consumed only by ACT exps and PE
matmul-sums, both read fp8 natively — the DVE, which has no fast path for
1-byte dtypes, never touches them). The mu tensors travel as bf16 (DVE 2x
mode). Per-core traffic: 2x2MB fp8 + 2x4MB bf16 = 12 MiB, vs the 358 GB/s
per-NeuronCore HBM limit -> ~37us stream.

Host packs row-contiguous [ROWS, 2, D] so each 128-row tile is ONE contiguous
line per partition -> 128 DMA descriptors per transfer (4x fewer than
plane-major; DMA_DIRECT2D issue measured 3.8us -> ~1us).

Work split sized to measured engine rates (DVE 0.96 GHz w/ 2x bf16 tensor-
tensor but 1x reduce, ACT 1.2 GHz, PE 0.55us per 512-col matmul, gpsimd
~40 G elem/s; tensor_tensor_reduce does not compile on this walrus build):
  DVE   : d = mq-mp ; dd = d*d ; dd[0:W/2] += ep ; t = em*dd ;
          reduce t[:, 0:RV] -> acc_m
  gpsimd: dd[W/2:W] += ep  (the other half of the add)
  ACT   : em = exp(-sq) ; ep = exp(sp) ; Identity-accum t[:, RV:W] -> acc_a
  PE    : sum(sq) - sum(sp) via +/-1 fp8 stationaries into one [1,512] PSUM
Pipeline: 11 units (7 full 128-row tiles + 4 column-quarters of the last
tile), 6-slot ring, single HWDGE queue on SP, 2 DMAs per unit (sigma first
so ACT/PE start half a unit early).
"""

from contextlib import ExitStack

import numpy as np

import concourse.bass as bass
from concourse import mybir
from concourse.bass_utils import run_bass_kernel_spmd

B, D = 8192, 2048
NCORES = 8
ROWS = B // NCORES
P = 128
NT = ROWS // P  # 8 row-tiles
NQ = 4
NU = (NT - 1) + NQ  # 11 units
NSLOT = 6

F32 = mybir.dt.float32
BF16 = mybir.dt.bfloat16
FP8 = mybir.dt.float8e4


def _w_of(u):
    return D if u < NT - 1 else D // NQ


def _rv_of(w):
    # DVE reduce-X share of the t-sum; ACT Identity-accum takes the rest.
    return 960 if w == D else 240


def _build_nc(detect_races=True):
    nc = bass.Bass(
        trn_type="TRN2", target_bir_lowering=False,
        detect_race_conditions=detect_races,
    )

    xs = nc.dram_tensor("xs", [ROWS, 2, D], FP8, kind="ExternalInput")
    xm = nc.dram_tensor("xm", [ROWS, 2, D], BF16, kind="ExternalInput")
    out = nc.dram_tensor("out", [P, 3], F32, kind="ExternalOutput")

    Exp = mybir.ActivationFunctionType.Exp
    Identity = mybir.ActivationFunctionType.Identity
    Alu = mybir.AluOpType
    X = mybir.AxisListType.X

    ctx = ExitStack()
    with ctx:
        sslot = [
            ctx.enter_context(nc.sbuf_tensor(f"ss{k}", [P, 2 * D], FP8))
            for k in range(NSLOT)
        ]
        mslot = [
            ctx.enter_context(nc.sbuf_tensor(f"ms{k}", [P, 2 * D], BF16))
            for k in range(NSLOT)
        ]
        d_b = [ctx.enter_context(nc.sbuf_tensor(f"d{j}", [P, D], BF16)) for j in range(2)]
        dd_b = [ctx.enter_context(nc.sbuf_tensor(f"dd{j}", [P, D], BF16)) for j in range(2)]
        em_b = [ctx.enter_context(nc.sbuf_tensor(f"em{j}", [P, D], BF16)) for j in range(2)]
        ep_b = [ctx.enter_context(nc.sbuf_tensor(f"ep{j}", [P, D], BF16)) for j in range(2)]
        t_b = [ctx.enter_context(nc.sbuf_tensor(f"t{j}", [P, D], BF16)) for j in range(2)]
        acc_a = ctx.enter_context(nc.sbuf_tensor("acc_a", [P, NU], F32))
        acc_m = ctx.enter_context(nc.sbuf_tensor("acc_m", [P, NU], F32))
        res = ctx.enter_context(nc.sbuf_tensor("res", [P, 3], F32))
        ones8 = ctx.enter_context(nc.sbuf_tensor("ones8", [P, 1], FP8))
        neg8 = ctx.enter_context(nc.sbuf_tensor("neg8", [P, 1], FP8))
        psAB = ctx.enter_context(nc.psum_tensor("psAB", [P, 512], F32))

        dsem = ctx.enter_context(nc.semaphore("dsem"))
        v_sem = ctx.enter_context(nc.semaphore("v_sem"))
        a_sem = ctx.enter_context(nc.semaphore("a_sem"))
        p_sem = ctx.enter_context(nc.semaphore("p_sem"))
        g_sem = ctx.enter_context(nc.semaphore("g_sem"))
        g2_sem = ctx.enter_context(nc.semaphore("g2_sem"))
        aa_sem = ctx.enter_context(nc.semaphore("aa_sem"))
        osem = ctx.enter_context(nc.semaphore("osem"))

        def src_ap(xt, u):
            if u < NT - 1:
                # one contiguous [2*D] line per row
                return bass.AP(xt, u * P * 2 * D, [[2 * D, P], [1, 2 * D]])
            q = u - (NT - 1)
            w = D // NQ
            # per row: both planes' q-th column slice
            return bass.AP(
                xt, (NT - 1) * P * 2 * D + q * w, [[2 * D, P], [D, 2], [1, w]]
            )

        with nc.Block() as block:

            @block.sync
            def _(sync):
                for u in range(NU):
                    if u >= NSLOT:
                        pu = u - NSLOT
                        sync.wait_ge(a_sem, 2 * pu + 2)  # ACT read sigma slot
                        sync.wait_ge(p_sem, pu + 1)      # PE read sigma slot
                        sync.wait_ge(v_sem, 3 * pu if pu else 1)  # DVE d-sub(pu)
                    w = _w_of(u)
                    k = u % NSLOT
                    sync.dma_start(sslot[k][:, 0 : 2 * w], src_ap(xs, u)).then_inc(
                        dsem, 16
                    )
                    sync.dma_start(mslot[k][:, 0 : 2 * w], src_ap(xm, u)).then_inc(
                        dsem, 16
                    )
                sync.wait_ge(v_sem, 3 * NU + 1)  # res final
                sync.dma_start(out[:, :], res[:, :]).then_inc(osem, 16)
                sync.wait_ge(osem, 16)

            @block.vector
            def _(vector):
                # Software-pipelined one unit deep: mul-t/red of unit u-1 run
                # between dd(u) (which unblocks gpsimd's add-right early) and
                # add-l(u). Keeps the gpsimd hop out of the DVE serial chain.
                def mul_red(x):
                    jx, wx = x % 2, _w_of(x)
                    rvx = _rv_of(wx)
                    if x >= 2:
                        vector.wait_ge(aa_sem, x - 1)  # t[jx] free (ACT accum x-2)
                    vector.wait_ge(g2_sem, x + 1)  # gpsimd add-right(x) done
                    vector.tensor_mul(
                        t_b[jx][:, 0:wx], em_b[jx][:, 0:wx], dd_b[jx][:, 0:wx]
                    ).then_inc(v_sem, 1)
                    vector.tensor_reduce(
                        acc_m[:, x : x + 1], t_b[jx][:, 0:rvx], axis=X, op=Alu.add
                    )

                for u in range(NU):
                    j, k, w = u % 2, u % NSLOT, _w_of(u)
                    ah = w // 2
                    vector.wait_ge(dsem, 32 * (u + 1))  # mu arrived (2nd DMA)
                    vector.tensor_sub(
                        d_b[j][:, 0:w], mslot[k][:, 0:w], mslot[k][:, w : 2 * w]
                    ).then_inc(v_sem, 1)
                    if u >= 2:
                        vector.wait_ge(g2_sem, u - 1)  # dd[j] free (gpsimd add u-2)
                    vector.tensor_mul(
                        dd_b[j][:, 0:w], d_b[j][:, 0:w], d_b[j][:, 0:w]
                    ).then_inc(v_sem, 1)
                    if u >= 1:
                        mul_red(u - 1)
                    vector.wait_ge(a_sem, 2 * u + 2)  # ep ready
                    vector.tensor_add(
                        dd_b[j][:, 0:ah], ep_b[j][:, 0:ah], dd_b[j][:, 0:ah]
                    )
                mul_red(NU - 1)
                vector.wait_ge(aa_sem, NU)  # all ACT accums final
                vector.tensor_reduce(res[:, 0:1], acc_a[:, 0:NU], axis=X, op=Alu.add)
                vector.tensor_reduce(res[:, 1:2], acc_m[:, 0:NU], axis=X, op=Alu.add)
                vector.wait_ge(p_sem, NU)  # all PE matmuls final
                vector.tensor_reduce(
                    res[0:1, 2:3], psAB[0:1, :], axis=X, op=Alu.add
                ).then_inc(v_sem, 1)

            @block.scalar
            def _(scalar):
                # The Identity-accum of unit u-1 runs AFTER em/ep of unit u:
                # lagging it one unit keeps the ep(u) -> gpsimd add(u) ->
                # DVE mul-t(u) -> accum(u) chain out of ACT's issue order, so
                # consecutive units pipeline instead of serializing.
                def id_accum(u):
                    j, w = u % 2, _w_of(u)
                    rv = _rv_of(w)
                    # mul-t(u) is DVE-emitted in iteration u+1 (count 3u+5),
                    # except the last unit's, which is the epilogue (3*NU).
                    tcnt = 3 * u + 5 if u < NU - 1 else 3 * NU
                    scalar.wait_ge(v_sem, tcnt)  # t(u) ready
                    scalar.activation(
                        t_b[j][:, rv:w], t_b[j][:, rv:w], Identity,
                        accum_out=acc_a[:, u : u + 1],
                    ).then_inc(aa_sem, 1)

                for u in range(NU):
                    j, k, w = u % 2, u % NSLOT, _w_of(u)
                    scalar.wait_ge(dsem, 32 * u + 16)  # sigma arrived (1st DMA)
                    if u >= 2:
                        scalar.wait_ge(v_sem, 3 * u - 1)  # em/ep free (mul-t(u-2))
                        scalar.wait_ge(g2_sem, u - 1)  # ep free (gpsimd add u-2)
                    scalar.activation(
                        em_b[j][:, 0:w], sslot[k][:, 0:w], Exp, scale=-1.0
                    ).then_inc(a_sem, 1)
                    scalar.activation(
                        ep_b[j][:, 0:w], sslot[k][:, w : 2 * w], Exp
                    ).then_inc(a_sem, 1)
                    if u >= 1:
                        id_accum(u - 1)
                id_accum(NU - 1)

            @block.tensor
            def _(pe):
                pe.wait_ge(g_sem, 1)  # stationary vectors ready
                mm = 0
                for u in range(NU):
                    k, w = u % NSLOT, _w_of(u)
                    nch = max(1, w // 512)
                    cw = min(w, 512)
                    pe.wait_ge(dsem, 32 * u + 16)  # sigma arrived
                    for c in range(nch):
                        pe.matmul(
                            psAB[0:1, 0:cw], ones8[:, 0:1],
                            sslot[k][:, c * cw : (c + 1) * cw],
                            start=(mm == 0), stop=False,
                        )
                        mm += 1
                    for c in range(nch):
                        i = pe.matmul(
                            psAB[0:1, 0:cw], neg8[:, 0:1],
                            sslot[k][:, w + c * cw : w + (c + 1) * cw],
                            start=False, stop=(u == NU - 1 and c == nch - 1),
                        )
                        mm += 1
                    i.then_inc(p_sem, 1)

            @block.gpsimd
            def _(gpsimd):
                gpsimd.memset(res[:, :], 0.0)
                gpsimd.memset(ones8[:, :], 1.0)
                gpsimd.memset(neg8[:, :], -1.0).then_inc(g_sem, 1)
                for u in range(NU):
                    j, w = u % 2, _w_of(u)
                    ah = w // 2
                    gpsimd.wait_ge(a_sem, 2 * u + 2)  # ep ready
                    gpsimd.wait_ge(v_sem, (3 * u + 1) if u else 2)  # dd written
                    gpsimd.tensor_add(
                        dd_b[j][:, ah:w], ep_b[j][:, ah:w], dd_b[j][:, ah:w]
                    ).then_inc(g2_sem, 1)

    return nc


_NC = None


def _get_nc():
    global _NC
    if _NC is None:
        _NC = _build_nc()
    return _NC


def _pack_inputs(inputs):
    fp8 = np.dtype(mybir.dt.np(FP8))
    bf16 = np.dtype(mybir.dt.np(BF16))
    xs = np.stack(
        [
            np.asarray(inputs["sigma_q"], dtype=np.float32),
            np.asarray(inputs["sigma_p"], dtype=np.float32),
        ],
        axis=1,
    ).astype(fp8)  # [B, 2, D]
    xm = np.stack(
        [
            np.asarray(inputs["mu_q"], dtype=np.float32),
            np.asarray(inputs["mu_p"], dtype=np.float32),
        ],
        axis=1,
    ).astype(bf16)  # [B, 2, D]
    return [
        {
            "xs": np.ascontiguousarray(xs[c * ROWS : (c + 1) * ROWS]),
            "xm": np.ascontiguousarray(xm[c * ROWS : (c + 1) * ROWS]),
        }
        for c in range(NCORES)
    ]


def _run(inputs, **kw):
    return run_bass_kernel_spmd(
        _get_nc(), _pack_inputs(inputs), core_ids=list(range(NCORES)), **kw
    )


def _combine(results):
    # per core: res[:,0]=acc_a rowsums, res[:,1]=acc_m rowsums,
    #           res[0,2]=sum(sq)-sum(sp)
    tot = 0.0
    for r in results:
        o = np.asarray(r["out"], dtype=np.float64)
        tot += o[:, 0].sum() + o[:, 1].sum() + o[0, 2]
    kl = 0.5 * (tot - B * D)
    return np.asarray(kl, dtype=np.float32)


def kernel(**inputs):
    return _combine(_run(inputs).results)


def run_traced(inputs, **kw):
    br = _run(inputs, trace=True, **kw)
    return _combine(br.results), br
